# revision 1
# baseline (speedup 1.0000x reference)
"""NT-Xent loss on 8 Trainium2 NeuronCores — fp8 DoubleRow edition.

Math (reference): xn = row-normalized x; mat = exp(xn @ xn.T / 0.1) with zero
diagonal; numer_r = mat[r, r±B]; denom_r = column sum r; loss = -mean(log(numer/denom)).

Because mat is symmetric, column sums equal row sums, so a core that owns a
row block [1024, 8192] computes its denominators entirely locally — no
collectives.  Each core c receives x rolled by -1024*c rows so that, in its
local column coordinates, the diagonal sits at col j'=i and the positive pair
at col j'=4096+i for local row i: the special tiles land at the same
compile-time position on every core, keeping the program SPMD-uniform.

The host prepares the operand layout (same species as the per-core roll): it
row-normalizes x, quantizes to fp8 e4m3, and lays it out pre-transposed as
xnT[p, k, i] = xn_fp8[i, 128k + p].  On-device work is then exactly the
similarity matmuls + exp + row sums:

  1. Each 128-row block of the strip is computed left-to-right in six column
     chunks of widths [1536, 1536, 1024, 1536, 1536, 1024].  A chunk is
     fp8 DoubleRow matmuls (2 packed k-subtiles, 512-col regions) into PSUM.
  2. The 1536-wide chunks drain on ACT — one Exp activation with accum_out
     producing the chunk row-sum — on a two-buffer (3+3 bank) psum
     ping-pong whose reuse distance (one 1536 + one 1024 fill) exceeds the
     ACT drain latency, so PE never stalls on ACT.  The 1024-wide chunks
     drain on DVE into their own 2-bank psum buffer, using a Schraudolph
     exp: i32 = round(A*s+B) bit-cast to f32 is 2^(10*log2(e)*s) with ~1.8%
     per-element noise that is mean-calibrated (constant C below) and
     averages out in the denominator sums.  The diagonal (col chunk 0) and
     the positive pair (col 4096, start of chunk 3) always land in exact
     ACT chunks.
  3. Diagonal / positive values are extracted on DVE (multiply by identity,
     reduce) per row block, always from an exact-exp chunk.
  4. The raw per-chunk row sums, diagonal and positive values ship in one
     output DMA; the host finishes denom = sum - diag, log and the mean in
     float64.
"""

import functools
import math

import ml_dtypes
import numpy as np

N, D, B = 8192, 512, 4096
NCORES = 8
RPC = N // NCORES           # 1024 local rows per core
MB = RPC // 128             # 8 row blocks of 128
KT = D // 128               # 4 contraction subtiles (2 DoubleRow pairs)
TEMP_INV = 10.0             # 1 / temperature

# Column chunks per row block: (start, width, drain engine).  The last row
# block ends on two 512-wide chunks so the final drains are short.
CHUNKS = [(0, 1536, "act"), (1536, 1536, "act"), (3072, 1024, "dve"),
          (4096, 1536, "act"), (5632, 1536, "act"), (7168, 1024, "dve")]
CHUNKS_LAST = [(0, 1536, "act"), (1536, 1536, "act"), (3072, 1024, "dve"),
               (4096, 1536, "act"), (5632, 1536, "act"), (7168, 512, "dve"),
               (7680, 512, "act")]
DIAG_CHUNK = 0              # self-similarity at col m*128+p -> chunk 0 (ACT)
NUMER_CHUNK = 3             # positive pair at col 4096+i -> chunk 3 (ACT)


def _schedule():
    """Flat (m, chunk) emission order.  The first three row blocks are
    half-row interleaved so the early chunks re-read the DMA slices already
    on chip while the rest of the input streams in."""
    sched = []
    for m in (0, 1, 2):
        sched += [(m, ci) for ci in (0, 1, 2)]
    for m in (0, 1, 2):
        sched += [(m, ci) for ci in (3, 4, 5)]
    for m in range(3, MB - 1):
        sched += [(m, ci) for ci in range(len(CHUNKS))]
    sched += [(MB - 1, ci) for ci in range(len(CHUNKS_LAST))]
    return sched


SCHEDULE = _schedule()
NRS = len(SCHEDULE)         # one row-sum slot per scheduled chunk

# Schraudolph exp: bitcast_f32(i32(A*s + B)) ~= exp(10*s).  C calibrated to
# zero the mean multiplicative error over the similarity distribution.
SCH_C = 480111.27
SCH_A = float(2**23 * TEMP_INV / math.log(2.0))
SCH_B = float(127.0 * 2**23 - SCH_C)



def _build():
    from contextlib import ExitStack

    import concourse.bacc as bacc
    import concourse.mybir as mybir
    import concourse.tile as tile

    F32 = mybir.dt.float32
    F8 = mybir.dt.float8e4
    I32 = mybir.dt.int32
    ALU = mybir.AluOpType
    ACTF = mybir.ActivationFunctionType
    AX = mybir.AxisListType
    DR = mybir.MatmulPerfMode.DoubleRow

    nc = bacc.Bacc("TRN2", target_bir_lowering=False, debug=False,
                   num_devices=NCORES)
    U8 = mybir.dt.uint8
    # uint8 carrier for the fp8 payload: fp8 NEFF i/o dtypes are flaky on
    # the PJRT transfer path; bitcast to f8 at the matmul operands instead.
    # Pre-sliced on the host so each 1024-col slice transfers contiguously.
    xnT_in = nc.dram_tensor("xnT", [N // 1024, 128, KT, 1024], U8,
                            kind="ExternalInput").ap()
    eye32_in = nc.dram_tensor("eye32", [128, 128], F32, kind="ExternalInput").ap()
    # One shipped block: per-chunk row sums [NRS] | diag [8] | numer [8];
    # the host finishes denom = sum - diag in float64.
    out_nd = nc.dram_tensor("numden", [128, NRS + 2 * MB], F32,
                            kind="ExternalOutput").ap()

    with ExitStack() as ctx:
        tc = ctx.enter_context(tile.TileContext(nc))
        consts = ctx.enter_context(tc.tile_pool(name="consts", bufs=1))
        xnp = ctx.enter_context(tc.tile_pool(name="xn", bufs=1))
        stats = ctx.enter_context(tc.tile_pool(name="stats", bufs=1))
        jact = ctx.enter_context(tc.tile_pool(name="jact", bufs=2))
        jdve = ctx.enter_context(tc.tile_pool(name="jdve", bufs=2))
        pst = ctx.enter_context(tc.tile_pool(name="pst", bufs=1, space="PSUM"))

        # Trigger the exp table load while the input DMA streams.
        warm = consts.tile([128, 1], F32, tag="warm")
        wjunk = consts.tile([128, 1], F32, tag="wjunk")
        nc.gpsimd.memset(warm[:], 0.0)
        nc.scalar.activation(wjunk[:], warm[:], ACTF.Exp)

        eye32 = consts.tile([128, 128], F32, tag="eye32")

        # Column-sliced input tiles: 1024 cols each; the first two slices
        # arrive as 512-col pieces so matmuls start as early as possible.
        xt = [xnp.tile([128, KT, 1024], U8, tag=f"xt{j}", name=f"xt{j}")
              for j in range(N // 1024)]
        nc.sync.dma_start(xt[0][:, :, 0:512], xnT_in[0, :, :, 0:512])
        nc.sync.dma_start(xt[0][:, :, 512:1024], xnT_in[0, :, :, 512:1024])
        for j in range(1, N // 1024):
            nc.sync.dma_start(xt[j][:], xnT_in[j])
        nc.gpsimd.dma_start(eye32[:], eye32_in)

        ship = stats.tile([128, NRS + 2 * MB], F32, tag="ship")
        rs = ship[:, 0:NRS]
        diagv = ship[:, NRS:NRS + MB]
        numv = ship[:, NRS + MB:NRS + 2 * MB]
        extj = stats.tile([128, 128], F32, tag="extj")

        # Dedicated psum buffers: ACT chunks ping-pong two 1536-wide (3-bank)
        # buffers; DVE chunks own a separate 1024-wide (2-bank) buffer.  The
        # pool's own rotation can hand consecutive chunks the same bank, so
        # allocate explicitly.
        psA = [pst.tile([128, 1536], F32, tag=f"psA{i}", name=f"psA{i}")
               for i in range(2)]
        psD = pst.tile([128, 1024], F32, tag="psD", name="psD")

        # HAM warm-up: a stream of dummy matmuls keeps the PE busy through
        # the initial DMA wait so the real matmuls start at the full clock
        # instead of the cold 4/8 throttle.
        wscr = consts.tile([128, 128], mybir.dt.bfloat16, tag="wscr")
        nc.vector.memset(wscr[:], 0.0)
        for _ in range(28):
            nc.tensor.matmul(psD[0:1, 0:128], lhsT=wscr[:, 0:1],
                             rhs=wscr[:], start=True, stop=True)

        n_act = 0
        for col, (m, ci) in enumerate(SCHEDULE):
            if True:
                g0, width, lane = (CHUNKS_LAST if m == MB - 1
                                   else CHUNKS)[ci]
                if lane == "act":
                    ps = psA[n_act % 2]
                    n_act += 1
                else:
                    ps = psD
                nreg = width // 512
                # k2-outer (one ldweights per k-pair); the very first chunk
                # goes region-major so it starts on the first 512-col DMA.
                if m == 0 and ci == 0:
                    order = [(r, k2) for r in range(nreg)
                             for k2 in range(KT // 2)]
                else:
                    order = [(r, k2) for k2 in range(KT // 2)
                             for r in range(nreg)]
                for r, k2 in order:
                    g = g0 + r * 512
                    nc.tensor.matmul(
                        ps[:, r * 512:(r + 1) * 512],
                        lhsT=xt[0][:, 2 * k2:2 * k2 + 2,
                                   m * 128:(m + 1) * 128].bitcast(F8),
                        rhs=xt[g // 1024][:, 2 * k2:2 * k2 + 2,
                                          g % 1024:g % 1024 + 512
                                          ].bitcast(F8),
                        start=(k2 == 0), stop=(k2 == KT // 2 - 1),
                        perf_mode=DR)
                if lane == "act":
                    if ci == DIAG_CHUNK or ci == NUMER_CHUNK:
                        eo = jact.tile([128, width], F32, tag="eo")
                        nc.scalar.activation(eo[:], ps[:, 0:width], ACTF.Exp,
                                             scale=TEMP_INV,
                                             accum_out=rs[:, col:col + 1])
                        # diag / positive cols (g0 + m*128 ..+128) sit at
                        # offset m*128 in this exact-exp chunk.
                        tgt = diagv if ci == DIAG_CHUNK else numv
                        # (tensor_tensor_reduce would fuse these two, but
                        # that opcode faults at runtime on this stack.)
                        nc.vector.tensor_mul(
                            extj[:], eo[:, m * 128:(m + 1) * 128],
                            eye32[:])
                        nc.vector.tensor_reduce(
                            tgt[:, m:m + 1], extj[:],
                            axis=AX.X, op=ALU.add)
                    else:
                        # exp in place over psum: ACT's PSUM port is cheaper
                        # than SBUF and no junk buffer is written.
                        nc.scalar.activation(ps[:, 0:width], ps[:, 0:width],
                                             ACTF.Exp, scale=TEMP_INV,
                                             accum_out=rs[:, col:col + 1])
                else:
                    ei = jdve.tile([128, width], I32, tag="ei")
                    nc.vector.tensor_scalar(ei[:], ps[:, 0:width],
                                            SCH_A, SCH_B,
                                            op0=ALU.mult, op1=ALU.add)
                    nc.vector.tensor_reduce(rs[:, col:col + 1],
                                            ei[:].bitcast(F32),
                                            axis=AX.X, op=ALU.add)

        nc.sync.dma_start(out_nd, ship[:])

    nc.finalize()
    return nc


@functools.lru_cache(maxsize=1)
def _get_nc():
    return _build()


def _prep_inputs(x):
    """Normalize + fp8-quantize + transpose to the DoubleRow operand layout."""
    x = np.asarray(x, dtype=np.float32)
    assert x.shape == (N, D)
    norm = np.linalg.norm(x, axis=1, keepdims=True)
    xn = x / np.maximum(norm, 1e-8)
    q = xn.astype(ml_dtypes.float8_e4m3)
    # xnT[p, k, i] = q[i, 128k + p]; shipped as the uint8 bit pattern,
    # pre-sliced into contiguous 1024-col blocks.
    layout = np.ascontiguousarray(q.T).reshape(KT, 128, N).transpose(1, 0, 2)
    layout = layout.view(np.uint8)
    eye32 = np.eye(128, dtype=np.float32)
    in_maps = []
    for c in range(NCORES):
        rolled = np.roll(layout, -c * RPC, axis=2)
        sliced = rolled.reshape(128, KT, N // 1024, 1024).transpose(2, 0, 1, 3)
        in_maps.append({"xnT": np.ascontiguousarray(sliced),
                        "eye32": eye32})
    return in_maps


def _run(x, **run_kwargs):
    from concourse.bass_utils import run_bass_kernel_spmd

    nc = _get_nc()
    return run_bass_kernel_spmd(nc, _prep_inputs(x), list(range(NCORES)),
                                **run_kwargs)


def _loss_from_results(results):
    cols_of_m = [[col for col, (m2, _) in enumerate(SCHEDULE) if m2 == m]
                 for m in range(MB)]
    nums, dens = [], []
    for c in range(NCORES):
        arr = results[c]["numden"].astype(np.float64)
        rowsum = np.stack([arr[:, cols].sum(axis=1) for cols in cols_of_m],
                          axis=1)
        diag = arr[:, NRS:NRS + MB]
        numer = arr[:, NRS + MB:NRS + 2 * MB]
        nums.append(numer.T.reshape(-1))
        dens.append((rowsum - diag).T.reshape(-1))
    num = np.concatenate(nums)
    den = np.concatenate(dens)
    loss = -np.sum(np.log(num / den)) / N
    return np.float32(loss)


def kernel(x):
    res = _run(x)
    return _loss_from_results(res.results)

